# revision 9
# baseline (speedup 1.0000x reference)
"""Trainium2 Bass kernel for nn_LossFunction_62852551409895 (topk_masking).

Computes: CE(outputs, labels) + sum_k CE(classifier[k], labels)
          + ALPHA * distance_loss(outputs, labels, ...)

Strategy: data-parallel over batch across 8 NeuronCores. Each core scans
its [4096, 1000] shard of each of the 3 heads once (memory-bound; the
stream runs at ~390 GB/s/core with 1-4MB transfers). Engine balance:

  - ACT (bottleneck): per row tile, ONE [128,2000] Exp over both
    classifier heads with a single accumulator (se12) + ONE [128,1000]
    Exp over the outputs head (se0). Accumulator reads land in PSUM.
  - DVE: InstMax (nc.vector.max) gives the top-8 of the outputs head in
    one 1-elem/cycle pass -> exact f32 v0/v1 (top_k tie semantics).
    DVE also reduce-sums esc1 (se1); host gets se2 = se12 - se1.
  - GpSimd: merged label gathers, one 64-wide (outputs head) and one
    128-wide (both classifier heads) indirect_copy per 4-tile macro with
    pre-offset indices; DVE diagonal-mask + reduce extracts own-label
    values.

The device ships a [128, 256] per-core stats tile (se1, xl0, xh, se0,
se12, v0/v1 per row tile); the host combine does the ln / distance-loss
branch math in f64 (compares in f32, so branch selection is bit-exact
vs the reference).
"""

import sys

for _p in ("/opt/trn_rl_repo", "/root/.axon_site/_ro/trn_rl_repo"):
    if _p not in sys.path:
        sys.path.append(_p)

from contextlib import ExitStack

import numpy as np

import concourse.bass as bass
import concourse.mybir as mybir
from concourse import bacc, tile
from concourse.bass_utils import run_bass_kernel_spmd

ALPHA = 0.1
B, C, K = 32768, 1000, 2
N_CORES = 8
R = B // N_CORES          # 4096 rows per core
P = 128                   # partitions
T = R // P                # 32 row tiles per core
J = 4                     # row tiles per macro
M = T // J                # 8 macros

F32 = mybir.dt.float32
U16 = mybir.dt.uint16
Alu = mybir.AluOpType
Act = mybir.ActivationFunctionType
AX = mybir.AxisListType

# out stats tile columns
O_SE1 = 0
O_XL0 = T
O_XH = 2 * T
O_SE0 = 4 * T
O_SE12 = 5 * T
O_V01 = 6 * T
O_W = 8 * T


def build_nc() -> bass.Bass:
    # Bacc (not raw Bass): its compile() pass splits semaphore waits to the
    # 1-per-instruction hardware limit (generate_event_semaphores).
    nc = bacc.Bacc("TRN2", target_bir_lowering=False)
    # [M, J, P, C] view of the row-major [R, C] shard: macro m covers rows
    # (4m+j)*128 + p.
    xout = nc.declare_dram_parameter("xout", [M, J, P, C], F32, isOutput=False)
    xcls = nc.declare_dram_parameter("xcls", [K, M, J, P, C], F32,
                                     isOutput=False)
    idxs = nc.declare_dram_parameter("idxs", [P, 12 * M], U16, isOutput=False)
    mask128 = nc.declare_dram_parameter("mask128", [P, P], F32, isOutput=False)
    res = nc.declare_dram_parameter("res", [P, O_W], F32, isOutput=True)

    with tile.TileContext(nc) as tc, ExitStack() as ctx:
        const_pool = ctx.enter_context(tc.tile_pool(name="const", bufs=1))
        d0_pool = ctx.enter_context(tc.tile_pool(name="d0", bufs=3))
        d12_pool = ctx.enter_context(tc.tile_pool(name="d12", bufs=2))
        esc0_pool = ctx.enter_context(tc.tile_pool(name="esc0", bufs=2))
        esc12_pool = ctx.enter_context(tc.tile_pool(name="esc12", bufs=4))
        g_pool = ctx.enter_context(tc.tile_pool(name="g", bufs=2))
        stats_pool = ctx.enter_context(tc.tile_pool(name="stats", bufs=1))
        acc_pool = ctx.enter_context(tc.psum_pool(name="acc", bufs=1))

        idx_t = const_pool.tile([P, 12 * M], U16)
        mask_t = const_pool.tile([P, P], F32)

        # ACT accumulator reads land in PSUM (ScE is closer to PSUM).
        se0S = acc_pool.tile([P, T], F32)      # ACT accum: sumexp head0
        se12S = acc_pool.tile([P, T], F32)     # ACT accum: sumexp h1+h2
        # All remaining per-row stats go straight into the DVE-written
        # output tile (single writer engine; one dma_start ships it).
        out_t = stats_pool.tile([P, O_W], F32)
        st8 = stats_pool.tile([P, 8 * T], F32)  # top-8 of head0 per tile

        for m in range(M):
            d0 = d0_pool.tile([P, J * C], F32, tag="d0")
            d12 = d12_pool.tile([P, J * 2 * C], F32, tag="d12")
            # d12 first (ACT's esc12 is the longest per-sub-tile op and
            # feeds DVE's se1), then d0. First/last macro use per-sub-tile
            # transfers (ramp/tail latency); middle macros use one 4MB
            # k-merged dma_start (fewer Sync dispatches).
            d12v = d12[:].rearrange("p (j h c) -> p j h c", h=2, c=C)
            d0v = d0[:].rearrange("p (j c) -> p j c", c=C)
            if m == 0:
                for j in range(J):
                    nc.sync.dma_start(
                        d12v[:, j], xcls[:, m, j].rearrange("k p c -> p k c")
                    )
                nc.sync.dma_start(d0v, xout[m].rearrange("j p c -> p j c"))
                # consts are first needed by macro-0 gathers; issuing them
                # after the data DMAs keeps the ramp free of small DMAs.
                nc.sync.dma_start(idx_t[:], idxs[:, :])
                nc.sync.dma_start(mask_t[:], mask128[:, :])
            elif m == M - 1:
                for j in range(J):
                    nc.sync.dma_start(
                        d12v[:, j], xcls[:, m, j].rearrange("k p c -> p k c")
                    )
                    nc.sync.dma_start(d0v[:, j], xout[m, j])
            else:
                nc.sync.dma_start(
                    d12v[:, :, 0, :], xcls[0, m].rearrange("j p c -> p j c")
                )
                nc.sync.dma_start(
                    d12v[:, :, 1, :], xcls[1, m].rearrange("j p c -> p j c")
                )
                nc.sync.dma_start(d0v, xout[m].rearrange("j p c -> p j c"))

            for j in range(J):
                t = m * J + j
                # ACT: esc12 first (DVE's se1 sum depends on it), then esc0
                # (dead store; only the accumulator value is used).
                esc12 = esc12_pool.tile([P, 2 * C], F32, tag="esc12")
                nc.scalar.activation(
                    esc12[:], d12[:, j * 2 * C:(j + 1) * 2 * C], Act.Exp,
                    accum_out=se12S[:, t:t + 1],
                )
                esc0 = esc0_pool.tile([P, C], F32, tag="esc0")
                nc.scalar.activation(
                    esc0[:], d0[:, j * C:(j + 1) * C], Act.Exp,
                    accum_out=se0S[:, t:t + 1],
                )
                # DVE: top-8 of head0 (exact f32 top-2 in one pass).
                nc.vector.max(st8[:, 8 * t:8 * t + 8], d0[:, j * C:(j + 1) * C])
                # DVE: sumexp of head1 from the live half of esc12.
                nc.vector.tensor_reduce(
                    out_t[:, O_SE1 + t:O_SE1 + t + 1], esc12[:, 0:C],
                    axis=AX.X, op=Alu.add,
                )

            # Label gathers, merged per macro (GpSimd indirect_copy):
            #   g0[p, s*16+q]  = d0 [p, idx[16*(p//16)+q, 12m+s]]     s=0..3
            #   g12[p, s*16+q] = d12[p, idx[16*(p//16)+q, 12m+4+s]]   s=0..7
            # idx values pre-offset: d0: j*C+lab; d12: j*2C+h*C+lab.
            g0 = g_pool.tile([P, 64], F32, tag="g0")
            nc.gpsimd.indirect_copy(
                g0[:], d0[:, :], idx_t[:, 12 * m:12 * m + 4], True,
            )
            g12 = g_pool.tile([P, 128], F32, tag="g12")
            nc.gpsimd.indirect_copy(
                g12[:], d12[:, :], idx_t[:, 12 * m + 4:12 * m + 12], True,
            )
            # Diagonal mask + per-16 reduce extracts own-label values.
            g0m = g_pool.tile([P, 64], F32, tag="g0m")
            nc.vector.scalar_tensor_tensor(
                g0m[:], g0[:], 1.0, mask_t[:, 0:64],
                op0=Alu.mult, op1=Alu.mult,
            )
            nc.vector.tensor_reduce(
                out_t[:, O_XL0 + J * m:O_XL0 + J * (m + 1)],
                g0m[:].rearrange("p (s q) -> p s q", q=16),
                axis=AX.X, op=Alu.add,
            )
            g12m = g_pool.tile([P, 128], F32, tag="g12m")
            nc.vector.scalar_tensor_tensor(
                g12m[:], g12[:], 1.0, mask_t[:, :],
                op0=Alu.mult, op1=Alu.mult,
            )
            nc.vector.tensor_reduce(
                out_t[:, O_XH + 8 * m:O_XH + 8 * (m + 1)],
                g12m[:].rearrange("p (s q) -> p s q", q=16),
                axis=AX.X, op=Alu.add,
            )

        # ---- Pack remaining stats and ship; loss math runs on host ----
        nc.vector.tensor_copy(out_t[:, O_SE0:O_SE0 + T], se0S[:])
        nc.vector.tensor_copy(out_t[:, O_SE12:O_SE12 + T], se12S[:])
        nc.vector.tensor_copy(
            out_t[:, O_V01:O_V01 + 2 * T].rearrange("p (t e) -> p t e", e=2),
            st8[:].rearrange("p (t e) -> p t e", e=8)[:, :, 0:2],
        )
        nc.sync.dma_start(res[:, :], out_t[:])

    nc.compile()
    return nc


def make_in_maps(outputs, outputs_classifier, labels):
    outputs = np.ascontiguousarray(np.asarray(outputs, dtype=np.float32))
    oc = np.ascontiguousarray(np.asarray(outputs_classifier, dtype=np.float32))
    labels = np.asarray(labels).astype(np.int64)

    # mask128[p, s*16+q] = (q == p % 16) for each 16-wide block s
    pp = np.arange(P)
    mask = np.zeros((P, P), dtype=np.float32)
    for s in range(8):
        mask[pp, s * 16 + (pp % 16)] = 1.0

    in_maps = []
    for c in range(N_CORES):
        lab_c = labels[c * R:(c + 1) * R].reshape(M, J, P)
        # idx columns per macro m (all 4B-aligned: 12m and 12m+4 are even):
        #   cols 12m   .. 12m+4  : d0 gather,  idx = j*C + lab        (s=j)
        #   cols 12m+4 .. 12m+12 : d12 gather, idx = j*2C + h*C + lab
        #     (s=0..3 -> h1 j=s, s=4..7 -> h2 j=s-4)
        idx = np.zeros((P, 12 * M), dtype=np.uint16)
        for m in range(M):
            for j in range(J):
                idx[:, 12 * m + j] = j * C + lab_c[m, j]
                idx[:, 12 * m + 4 + j] = j * 2 * C + lab_c[m, j]
                idx[:, 12 * m + 8 + j] = j * 2 * C + C + lab_c[m, j]
        in_maps.append({
            "xout": outputs[c * R:(c + 1) * R].reshape(M, J, P, C),
            "xcls": np.ascontiguousarray(
                oc[:, c * R:(c + 1) * R]).reshape(K, M, J, P, C),
            "idxs": idx,
            "mask128": mask,
        })
    return in_maps


_NC_CACHE = None


def get_nc():
    global _NC_CACHE
    if _NC_CACHE is None:
        _NC_CACHE = build_nc()
    return _NC_CACHE


def combine(results, weight_bias, args_bias, args_gamma):
    wb = np.asarray(weight_bias, dtype=np.float32)
    ab = np.asarray(args_bias, dtype=np.float32)
    ag = np.asarray(args_gamma, dtype=np.float32)
    th1, th2, b = (np.float64(wb[0]), np.float64(wb[1]), np.float64(wb[2]))
    inv_norm = 1.0 / np.sqrt(th1 * th1 + th2 * th2)
    gam = np.float64(ag[0])
    bc = b - np.float64(ab[0])

    ce_total = 0.0
    per_total = 0.0
    for r in results:
        o = r["res"]  # [P, 8T] f32
        se1 = o[:, O_SE1:O_SE1 + T]
        xl0 = o[:, O_XL0:O_XL0 + T]
        xh = o[:, O_XH:O_XH + 2 * T].reshape(P, M, 2, J)
        se0 = o[:, O_SE0:O_SE0 + T]
        se12 = o[:, O_SE12:O_SE12 + T]
        v01 = o[:, O_V01:O_V01 + 2 * T].reshape(P, T, 2)
        v0f, v1f = v01[:, :, 0], v01[:, :, 1]

        # branch selection compares in f32 (exact data values)
        ind = (xl0 >= v1f).astype(np.float64)

        se1_64 = se1.astype(np.float64)
        se2 = se12.astype(np.float64) - se1_64
        xl1 = xh[:, :, 0, :].reshape(P, T).astype(np.float64)
        xl2 = xh[:, :, 1, :].reshape(P, T).astype(np.float64)
        x = xl0.astype(np.float64)
        ce = (np.log(se0.astype(np.float64)) + np.log(se1_64) + np.log(se2)
              - (x + xl1 + xl2))
        ce_total += float(ce.sum())

        v0 = v0f.astype(np.float64)
        v1 = v1f.astype(np.float64)
        y = v0 + v1 - x * ind
        dist = (th1 * x + th2 * y + bc) * inv_norm
        per = np.where(dist >= 10.0, -2.0,
                       np.where(dist >= 0.0, -gam * dist, -dist))
        per_total += float(per.sum())

    return np.float32(ce_total / B + ALPHA * per_total)


def kernel(outputs, outputs_classifier, labels, weight_bias, args_bias,
           args_gamma) -> np.ndarray:
    nc = get_nc()
    in_maps = make_in_maps(outputs, outputs_classifier, labels)
    results = run_bass_kernel_spmd(nc, in_maps, list(range(N_CORES))).results
    return np.array(
        combine(results, weight_bias, args_bias, args_gamma), dtype=np.float32
    )


if __name__ == "__main__":
    d = np.load("/tmp/inputs_cache.npz")
    out = kernel(**{k: d[k] for k in d.files})
    print("kernel output:", out)
    ref = np.load("/tmp/ref_value.npy")
    print("reference:    ", ref)
    print("rel err:      ", abs(float(out) - float(ref)) / abs(float(ref)))
